# revision 5
# baseline (speedup 1.0000x reference)
"""Trainium2 Bass kernel for nn_AttnBlockpp3d_old (GroupNorm + 4-head spatial
self-attention + residual), data-parallel over batch across 8 NeuronCores.

Shapes (hardcoded): x [16, 256, 32, 32] f32 -> per core 2 batches of [256, 1024].

v2 design notes (vs the 153us baseline):
- ScalarE is the near-critical path (softmax exp). It runs ONLY Exp
  activations (one table set, loaded once via an early dummy): 1024-wide
  exps at (N+352)/1.2ns amortize the per-instruction overhead that made the
  512-wide baseline pay 92us (64 exps x 1147ns ~= 73us now).
- All nin biases are eliminated from the device hot path by host-side algebra:
  b1 (k bias) cancels exactly in softmax; b2 (v bias) folds into b3
  (b3' = b3 + W3^T b2, since softmax weights sum to 1); b0 (q bias) becomes a
  per-key-position score bias g(kp) = k0^T b0, computed for free as 4 extra
  output columns of the v-projection matmul (W1 @ b0 per head appended to the
  W2 stationary), applied via the Exp activation's per-partition bias AP.
- GroupNorm rsqrt via quake-rsqrt bit tricks on VectorE (no Ln/Exp on
  ScalarE -> no activation-table switches; baseline lost 5us to 4 table loads).
- x loaded once (stats run on the [128,1024] ct-tiles directly; groups don't
  cross the ct boundary), W pre-packed bf16 on host (halves weight DMA).
- Softmax denominator rides A@V as a vt ones-column (baseline trick), but the
  partition-broadcast of 1/denom is a K=1 ones-matmul of the bf16 denom row +
  reciprocal AFTER broadcast (all lanes parallel) -- no DRAM bounce; the
  baseline's bounce serialized a ~10us tail.
- Per-pr j-outer pipeline: scores (2x512) -> 1024-wide exp -> lag-1 A@V, with
  PSUM tags sc (2 tiles x 2 banks) + hh (4 x 1 bank) fitting 8 banks.
"""

import numpy as np

N_CORES = 8
B_TOTAL = 16
B_PER_CORE = B_TOTAL // N_CORES
C = 256
H = 32
S = H * H          # 1024 spatial positions
NG = 32            # groupnorm groups -> 8 channels/group
NH = 4             # heads
CH = C // NH       # 64 channels/head
EPS = 1e-6
SCALE = CH ** -0.5  # 0.125

_CACHE: dict = {}


def _build_nc():
    from contextlib import ExitStack

    import concourse.bacc as bacc
    import concourse.bass as bass
    import concourse.mybir as mybir
    import concourse.tile as tile

    fp32 = mybir.dt.float32
    bf16 = mybir.dt.bfloat16
    i32 = mybir.dt.int32
    AF = mybir.ActivationFunctionType
    OP = mybir.AluOpType
    ts = bass.ts

    nc = bacc.Bacc("TRN2")

    x_d = nc.dram_tensor("x", [B_PER_CORE, C, S], fp32, kind="ExternalInput")
    wq_d = nc.dram_tensor("wq", [128, 2, C], bf16, kind="ExternalInput")
    wk_d = nc.dram_tensor("wk", [128, 2, C], bf16, kind="ExternalInput")
    wv_d = nc.dram_tensor("wv", [128, 2, C + NH], bf16, kind="ExternalInput")
    w3_d = nc.dram_tensor("w3", [128, 2, C], bf16, kind="ExternalInput")
    b3_d = nc.dram_tensor("b3p", [C], fp32, kind="ExternalInput")
    gns_d = nc.dram_tensor("gn_scale", [C], fp32, kind="ExternalInput")
    gnb_d = nc.dram_tensor("gn_bias", [C], fp32, kind="ExternalInput")
    y_d = nc.dram_tensor("y", [B_PER_CORE, C, S], fp32, kind="ExternalOutput")

    with tile.TileContext(nc) as tc, ExitStack() as ctx:
        const = ctx.enter_context(tc.tile_pool(name="const", bufs=1))
        xpool = ctx.enter_context(tc.tile_pool(name="xpool", bufs=1))
        opool = ctx.enter_context(tc.tile_pool(name="opool", bufs=2))
        hpool = ctx.enter_context(tc.tile_pool(name="hpool", bufs=2))
        qkpool = ctx.enter_context(tc.tile_pool(name="qkpool", bufs=1))
        vpool = ctx.enter_context(tc.tile_pool(name="vpool", bufs=16))
        gpool = ctx.enter_context(tc.tile_pool(name="gpool", bufs=16))
        epool = ctx.enter_context(tc.tile_pool(name="epool", bufs=6))
        upool = ctx.enter_context(tc.tile_pool(name="upool", bufs=2))
        npool = ctx.enter_context(tc.tile_pool(name="npool", bufs=4))
        spool = ctx.enter_context(tc.tile_pool(name="spool", bufs=2))
        ps = ctx.enter_context(tc.tile_pool(name="ps", bufs=1, space="PSUM"))

        # ---- phase 0: loads + constants ----
        x_sb = [[None, None], [None, None]]
        for ct in range(2):
            t = xpool.tile([128, S], fp32, tag=f"x0{ct}")
            nc.sync.dma_start(out=t, in_=x_d[0, ts(ct, 128), :])
            x_sb[0][ct] = t

        wq = const.tile([128, 2, C], bf16, tag="wq")
        nc.sync.dma_start(out=wq, in_=wq_d[:, :, :])
        wk = const.tile([128, 2, C], bf16, tag="wk")
        nc.sync.dma_start(out=wk, in_=wk_d[:, :, :])
        wv = const.tile([128, 2, C + NH], bf16, tag="wv")
        nc.sync.dma_start(out=wv, in_=wv_d[:, :, :])

        for ct in range(2):
            t = xpool.tile([128, S], fp32, tag=f"x1{ct}")
            nc.sync.dma_start(out=t, in_=x_d[1, ts(ct, 128), :])
            x_sb[1][ct] = t

        w3 = const.tile([128, 2, C], bf16, tag="w3")
        nc.sync.dma_start(out=w3, in_=w3_d[:, :, :])

        def col_tiles(dram, name):
            out = []
            for ct in range(2):
                t = const.tile([128, 1], fp32, tag=f"{name}{ct}")
                nc.sync.dma_start(out=t, in_=dram[ts(ct, 128)][:, None])
                out.append(t)
            return out

        gns_sb = col_tiles(gns_d, "gns")
        gnb_sb = col_tiles(gnb_d, "gnb")
        b3_sb = col_tiles(b3_d, "b3")

        # dummy exp: forces the single ACT table load before the stream
        dmy = spool.tile([1, 8], fp32, tag="dmy")
        nc.vector.memset(dmy, 0.0)
        nc.scalar.activation(out=dmy, in_=dmy, func=AF.Exp, scale=1.0)

        # ones row for the K=1 denominator-broadcast matmul
        ones1 = const.tile([1, 64], bf16, tag="ones1")
        nc.gpsimd.memset(ones1, 1.0)

        # HAM warm-up: keep PE busy through the load phase
        warm = const.tile([128, 512], bf16, tag="warm")
        nc.vector.memset(warm, 1.0)
        for i in range(28):
            warm_ps = ps.tile([128, 512], fp32, tag="hh", bufs=4, name="warm_ps")
            nc.tensor.matmul(warm_ps, lhsT=warm[:, 0:128], rhs=warm,
                             start=True, stop=True)

        # q1[ct] [128, 32]: q1[p, g] = 1 iff group(ct*128+p) == g  (p//8 == g-16ct)
        q1 = []
        for ct in range(2):
            t = const.tile([128, NG], fp32, tag=f"q1{ct}")
            nc.gpsimd.memset(t, 1.0)
            nc.gpsimd.affine_select(out=t, in_=t, compare_op=OP.is_ge, fill=0.0,
                                    pattern=[[-8, NG]], base=128 * ct,
                                    channel_multiplier=1)
            nc.gpsimd.affine_select(out=t, in_=t, compare_op=OP.is_ge, fill=0.0,
                                    pattern=[[8, NG]], base=7 - 128 * ct,
                                    channel_multiplier=-1)
            q1.append(t)

        # q2[ct] [32, 128]: q2[g, c] = 1 iff group(ct*128+c) == g
        q2 = []
        for ct in range(2):
            t = const.tile([NG, 128], fp32, tag=f"q2{ct}")
            nc.gpsimd.memset(t, 1.0)
            base = ct * 128
            nc.gpsimd.affine_select(out=t, in_=t, compare_op=OP.is_ge, fill=0.0,
                                    pattern=[[1, 128]], base=base,
                                    channel_multiplier=-8)
            nc.gpsimd.affine_select(out=t, in_=t, compare_op=OP.is_ge, fill=0.0,
                                    pattern=[[-1, 128]], base=7 - base,
                                    channel_multiplier=8)
            q2.append(t)

        # ---- phase 1 per batch: groupnorm + q/k/vT projections ----
        # E [65, 64]: row 64 = 1 else 0 (denominator-broadcast stationary)
        E = const.tile([65, 64], bf16, tag="E")
        nc.gpsimd.memset(E, 0.0)
        nc.gpsimd.memset(E[64:65, :], 1.0)

        qk_sb_all, vt_all, gsb_all = [], [], []

        def phase1(b):
            # stats per ct-tile (groups don't cross the ct boundary)
            gs_ps = ps.tile([NG, 2], fp32, tag="s0", bufs=1, name="gs_ps")
            for ct in range(2):
                st6 = spool.tile([128, 2, 6], fp32, tag="st6")
                for k in range(2):
                    nc.vector.bn_stats(out=st6[:, k, :],
                                       in_=x_sb[b][ct][:, ts(k, 512)])
                mv = spool.tile([128, 2], fp32, tag="mv")
                nc.vector.bn_aggr(out=mv, in_=st6)
                rhs2 = spool.tile([128, 2], fp32, tag="rhs2")
                nc.vector.tensor_copy(out=rhs2[:, 0:1], in_=mv[:, 0:1])
                nc.vector.tensor_mul(out=rhs2[:, 1:2], in0=mv[:, 0:1],
                                     in1=mv[:, 0:1])
                nc.vector.tensor_add(out=rhs2[:, 1:2], in0=rhs2[:, 1:2],
                                     in1=mv[:, 1:2])
                nc.tensor.matmul(gs_ps, lhsT=q1[ct], rhs=rhs2,
                                 start=(ct == 0), stop=(ct == 1))
            gmv = spool.tile([NG, 2], fp32, tag="gmv")
            nc.vector.tensor_scalar_mul(out=gmv, in0=gs_ps, scalar1=1.0 / 8.0)
            # v = var + eps
            v = spool.tile([NG, 1], fp32, tag="qv")
            nc.vector.tensor_mul(out=v, in0=gmv[:, 0:1], in1=gmv[:, 0:1])
            nc.vector.tensor_tensor(out=v, in0=gmv[:, 1:2], in1=v,
                                    op=OP.subtract)
            nc.vector.tensor_scalar_add(out=v, in0=v, scalar1=EPS)
            # quake rsqrt + 2 Newton steps
            t1 = spool.tile([NG, 1], i32, tag="qt1")
            nc.vector.tensor_scalar(out=t1, in0=v.bitcast(i32), scalar1=1,
                                    scalar2=0xFFFFFFFF,
                                    op0=OP.logical_shift_right,
                                    op1=OP.bitwise_xor)
            y0i = spool.tile([NG, 1], i32, tag="qy0")
            nc.vector.tensor_scalar(out=y0i, in0=t1, scalar1=0x5f3759e0,
                                    scalar2=None, op0=OP.add)
            ab_g = spool.tile([NG, 2], fp32, tag="abg")
            y2 = spool.tile([NG, 1], fp32, tag="qy2")
            t3 = spool.tile([NG, 1], fp32, tag="qt3")
            y1 = spool.tile([NG, 1], fp32, tag="qy1")
            cur = y0i.bitcast(fp32)
            for it in range(2):
                nc.vector.tensor_mul(out=y2, in0=cur, in1=cur)
                nc.vector.tensor_mul(out=t3, in0=y2, in1=v)
                nc.vector.tensor_scalar(out=t3, in0=t3, scalar1=-0.5,
                                        scalar2=1.5, op0=OP.mult, op1=OP.add)
                dst = ab_g[:, 0:1] if it == 1 else y1
                nc.vector.tensor_mul(out=dst, in0=cur, in1=t3)
                cur = dst
            nc.vector.tensor_mul(out=ab_g[:, 1:2], in0=gmv[:, 0:1],
                                 in1=ab_g[:, 0:1])
            nc.vector.tensor_scalar_mul(out=ab_g[:, 1:2], in0=ab_g[:, 1:2],
                                        scalar1=-1.0)

            h_bf = []
            for ct in range(2):
                ab_ps = ps.tile([128, 2], fp32, tag="s1", bufs=1, name="ab_ps")
                nc.tensor.matmul(ab_ps, lhsT=q2[ct], rhs=ab_g, start=True,
                                 stop=True)
                AB = spool.tile([128, 2], fp32, tag=f"AB{ct}")
                nc.vector.tensor_mul(out=AB[:, 0:1], in0=ab_ps[:, 0:1],
                                     in1=gns_sb[ct])
                nc.vector.tensor_mul(out=AB[:, 1:2], in0=ab_ps[:, 1:2],
                                     in1=gns_sb[ct])
                nc.vector.tensor_add(out=AB[:, 1:2], in0=AB[:, 1:2],
                                     in1=gnb_sb[ct])
                ht = hpool.tile([128, S], bf16, tag=f"h{ct}")
                nc.vector.tensor_scalar(out=ht, in0=x_sb[b][ct],
                                        scalar1=AB[:, 0:1], scalar2=AB[:, 1:2],
                                        op0=OP.mult, op1=OP.add)
                h_bf.append(ht)
            # residual tile absorbs b3' (x + b3' + W3 hh_n is the output)
            for ct in range(2):
                nc.vector.tensor_scalar_add(out=x_sb[b][ct], in0=x_sb[b][ct],
                                            scalar1=b3_sb[ct])

            # q/k projections -> bf16 [d_tile 128, s 1024]
            qk_sb = [[None, None], [None, None]]
            for p, wt in ((0, wq), (1, wk)):
                for dt in range(2):
                    qk_ps = ps.tile([128, S], fp32, tag=f"s{(2 * p + dt) % 2}",
                                    bufs=1, name="qk_ps")
                    for sc in range(2):
                        for ct in range(2):
                            nc.tensor.matmul(
                                qk_ps[:, ts(sc, 512)],
                                lhsT=wt[:, ct, ts(dt, 128)],
                                rhs=h_bf[ct][:, ts(sc, 512)],
                                start=(ct == 0), stop=(ct == 1))
                    t = qkpool.tile([128, S], bf16, tag=f"qk{b}{p}{dt}")
                    nc.vector.tensor_copy(out=t, in_=qk_ps)
                    qk_sb[p][dt] = t

            # vT tiles [128 kp, 4 heads, 65] + per-head g columns
            vt_tiles, gsb_tiles = [], []
            for j in range(8):
                vt_ps = ps.tile([128, C + NH], fp32, tag=f"s{j % 2}",
                                bufs=1, name="vt_ps")
                for ct in range(2):
                    nc.tensor.matmul(vt_ps, lhsT=h_bf[ct][:, ts(j, 128)],
                                     rhs=wv[:, ct, :], start=(ct == 0),
                                     stop=(ct == 1))
                vt = vpool.tile([128, NH, CH + 1], bf16, tag="vt")
                nc.gpsimd.memset(vt[:, :, CH:CH + 1], 1.0)
                nc.vector.tensor_copy(
                    out=vt[:, :, 0:CH],
                    in_=vt_ps[:, 0:C].rearrange("p (h c) -> p h c", h=NH))
                gsb = gpool.tile([128, NH], fp32, tag="gsb")
                nc.vector.tensor_scalar_mul(out=gsb, in0=vt_ps[:, C:C + NH],
                                            scalar1=SCALE)
                vt_tiles.append(vt)
                gsb_tiles.append(gsb)
            qk_sb_all.append(qk_sb)
            vt_all.append(vt_tiles)
            gsb_all.append(gsb_tiles)

        phase1(0)
        phase1(1)

        # ---- phase 2: attention pr-blocks, software-pipelined ----
        blocks = [(0, 0), (0, 1), (1, 0), (1, 1)]
        hh_n_all = {}

        def scores_exp(b, pr, j, hp, s_tiles):
            qk_sb = qk_sb_all[b]
            s_ps = ps.tile([128, S], fp32, tag=f"s{hp}", bufs=1, name="s_ps")
            for sc in range(2):
                nc.tensor.matmul(
                    s_ps[:, ts(sc, 512)],
                    lhsT=qk_sb[1][pr][ts(hp, CH), ts(j, 128)],
                    rhs=qk_sb[0][pr][ts(hp, CH), ts(sc, 512)],
                    start=True, stop=True)
            et = epool.tile([128, S], bf16, tag="e")
            nc.scalar.activation(out=et, in_=s_ps, func=AF.Exp,
                                 bias=gsb_all[b][j][:, 2 * pr + hp:2 * pr + hp + 1],
                                 scale=SCALE)
            s_tiles[(j, hp)] = et

        def av(b, pr, j, hp, s_tiles, hh_ps):
            vt = vt_all[b][j]
            et = s_tiles[(j, hp)]
            for sc in range(2):
                nc.tensor.matmul(hh_ps[hp][sc],
                                 lhsT=vt[:, 2 * pr + hp, :],
                                 rhs=et[:, ts(sc, 512)],
                                 start=(j == 0), stop=(j == 7))

        def normalize(b, pr, hp, hh_ps):
            # bf16 eviction feeds the E-matmul denom broadcast; numerator
            # stays fp32 in PSUM for the final multiply.
            hh_b = upool.tile([CH + 1, S], bf16, tag="hhb")
            rd = upool.tile([CH, S], fp32, tag="rd")
            hh_n = hh_n_all[(b, pr)]
            for sc in range(2):
                nc.vector.tensor_copy(out=hh_b[:, ts(sc, 512)],
                                      in_=hh_ps[hp][sc])
            db_ps = [ps.tile([CH, 512], fp32, tag=f"s{sc}", bufs=1,
                             name=f"db{sc}") for sc in range(2)]
            for sc in range(2):
                nc.tensor.matmul(db_ps[sc], lhsT=E,
                                 rhs=hh_b[:, ts(sc, 512)], start=True,
                                 stop=True)
            for sc in range(2):
                nc.vector.reciprocal_approx_fast(out=rd[:, ts(sc, 512)],
                                                 in_=db_ps[sc])
            for sc in range(2):
                nc.vector.tensor_mul(out=hh_n[ts(hp, CH), ts(sc, 512)],
                                     in0=hh_ps[hp][sc][0:CH, :],
                                     in1=rd[:, ts(sc, 512)])

        def fin(b, dt):
            # final nin + residual + output DMA for one d-tile
            fin_ps = ps.tile([128, S], fp32, tag=f"s{dt}", bufs=1,
                             name="fin_ps")
            for sc in range(2):
                for ct in range(2):
                    nc.tensor.matmul(
                        fin_ps[:, ts(sc, 512)],
                        lhsT=w3[:, ct, ts(dt, 128)],
                        rhs=hh_n_all[(b, ct)][:, ts(sc, 512)],
                        start=(ct == 0), stop=(ct == 1))
            out_t = opool.tile([128, S], fp32, tag=f"out{dt}")
            nc.vector.tensor_add(out=out_t, in0=fin_ps, in1=x_sb[b][dt])
            nc.sync.dma_start(out=y_d[b, ts(dt, 128), :], in_=out_t)

        prev = None
        prev_state = None
        for b, pr in blocks:
            hh_n_all[(b, pr)] = npool.tile([128, S], bf16, tag="hhn", name=f"hhn{b}{pr}")
            s_tiles = {}
            hh_ps = None
            for j in range(8):
                # lagged work from the previous block first (keeps the PSUM
                # WAR order correct: prev readers emitted before cur writers)
                if prev is not None:
                    pb, ppr = prev
                    if j == 0:
                        for hp in range(2):
                            av(pb, ppr, 7, hp, prev_state[0], prev_state[1])
                    elif j == 1:
                        for hp in range(2):
                            normalize(pb, ppr, hp, prev_state[1])
                    elif j == 2 and ppr == 1:
                        fin(pb, 0)
                    elif j == 3 and ppr == 1:
                        fin(pb, 1)
                for hp in range(2):
                    scores_exp(b, pr, j, hp, s_tiles)
                if j == 1:
                    hh_ps = [[ps.tile([CH + 1, 512], fp32, tag="hh", bufs=4,
                                      name=f"hh{hp}{sc}") for sc in range(2)]
                             for hp in range(2)]
                if j >= 1:
                    for hp in range(2):
                        av(b, pr, j - 1, hp, s_tiles, hh_ps)
            prev = (b, pr)
            prev_state = (s_tiles, hh_ps)

        # epilogue: last block's tail
        pb, ppr = prev
        for hp in range(2):
            av(pb, ppr, 7, hp, prev_state[0], prev_state[1])
        for hp in range(2):
            normalize(pb, ppr, hp, prev_state[1])
        fin(pb, 0)
        fin(pb, 1)

    nc.finalize()
    return nc


def _pack_weights(inputs):
    """Host-side algebra + bf16 packing. Returns dict of shared arrays."""
    import ml_dtypes

    W0 = np.asarray(inputs["W0"], np.float32)
    b0 = np.asarray(inputs["b0"], np.float32)
    W1 = np.asarray(inputs["W1"], np.float32)
    W2 = np.asarray(inputs["W2"], np.float32)
    b2 = np.asarray(inputs["b2"], np.float32)
    W3 = np.asarray(inputs["W3"], np.float32)
    b3 = np.asarray(inputs["b3"], np.float32)

    def pack(w):
        # [C, D] -> [128, 2, D] with c = ct*128 + p
        return np.ascontiguousarray(
            w.reshape(2, 128, -1).transpose(1, 0, 2)).astype(ml_dtypes.bfloat16)

    G = np.zeros((C, NH), np.float32)
    for h in range(NH):
        G[:, h] = W1[:, h * CH:(h + 1) * CH] @ b0[h * CH:(h + 1) * CH]
    wv_ext = np.concatenate([W2, G], axis=1)          # [C, 260]
    b3p = b3 + W3.T @ b2

    return {
        "wq": pack(W0),
        "wk": pack(W1),
        "wv": pack(wv_ext),
        "w3": pack(W3),
        "b3p": np.ascontiguousarray(b3p, np.float32),
        "gn_scale": np.ascontiguousarray(np.asarray(inputs["gn_scale"], np.float32)),
        "gn_bias": np.ascontiguousarray(np.asarray(inputs["gn_bias"], np.float32)),
    }


def _in_maps(inputs):
    x = np.ascontiguousarray(np.asarray(inputs["x"], dtype=np.float32))
    B = x.shape[0]
    xr = x.reshape(B, C, S)
    shared = _pack_weights(inputs)
    maps = []
    for core in range(N_CORES):
        m = dict(shared)
        m["x"] = np.ascontiguousarray(xr[core * B_PER_CORE:(core + 1) * B_PER_CORE])
        maps.append(m)
    return maps


def kernel(**inputs: np.ndarray) -> np.ndarray:
    from concourse.bass_utils import run_bass_kernel_spmd

    if "nc" not in _CACHE:
        _CACHE["nc"] = _build_nc()
    res = run_bass_kernel_spmd(_CACHE["nc"], _in_maps(inputs),
                               core_ids=list(range(N_CORES)))
    out = np.concatenate([res.results[c]["y"] for c in range(N_CORES)], axis=0)
    B = np.asarray(inputs["x"]).shape[0]
    return out.reshape(B, C, H, H).astype(np.float32)


def run_profiled(inputs):
    """Like kernel() but with trace=True; returns (out, exec_time_ns)."""
    from concourse.bass_utils import run_bass_kernel_spmd

    if "nc" not in _CACHE:
        _CACHE["nc"] = _build_nc()
    res = run_bass_kernel_spmd(_CACHE["nc"], _in_maps(inputs),
                               core_ids=list(range(N_CORES)), trace=True)
    out = np.concatenate([res.results[c]["y"] for c in range(N_CORES)], axis=0)
    B = np.asarray(inputs["x"]).shape[0]
    return out.reshape(B, C, H, H).astype(np.float32), res.exec_time_ns


# revision 6
# speedup vs baseline: 1.0820x; 1.0820x over previous
"""Trainium2 Bass kernel for nn_AttnBlockpp3d_old (GroupNorm + 4-head spatial
self-attention + residual), data-parallel over batch across 8 NeuronCores.

Shapes (hardcoded): x [16, 256, 32, 32] f32 -> per core 2 batches of [256, 1024].

v2 design notes (vs the 153us baseline):
- ScalarE is the near-critical path (softmax exp). It runs ONLY Exp
  activations (one table set, loaded once via an early dummy): 1024-wide
  exps at (N+352)/1.2ns amortize the per-instruction overhead that made the
  512-wide baseline pay 92us (64 exps x 1147ns ~= 73us now).
- All nin biases are eliminated from the device hot path by host-side algebra:
  b1 (k bias) cancels exactly in softmax; b2 (v bias) folds into b3
  (b3' = b3 + W3^T b2, since softmax weights sum to 1); b0 (q bias) becomes a
  per-key-position score bias g(kp) = k0^T b0, computed for free as 4 extra
  output columns of the v-projection matmul (W1 @ b0 per head appended to the
  W2 stationary), applied via the Exp activation's per-partition bias AP.
- GroupNorm rsqrt via quake-rsqrt bit tricks on VectorE (no Ln/Exp on
  ScalarE -> no activation-table switches; baseline lost 5us to 4 table loads).
- x loaded once (stats run on the [128,1024] ct-tiles directly; groups don't
  cross the ct boundary), W pre-packed bf16 on host (halves weight DMA).
- Softmax denominator rides A@V as a vt ones-column (baseline trick), but the
  partition-broadcast of 1/denom is a K=1 ones-matmul of the bf16 denom row +
  reciprocal AFTER broadcast (all lanes parallel) -- no DRAM bounce; the
  baseline's bounce serialized a ~10us tail.
- Per-pr j-outer pipeline: scores (2x512) -> 1024-wide exp -> lag-1 A@V, with
  PSUM tags sc (2 tiles x 2 banks) + hh (4 x 1 bank) fitting 8 banks.
"""

import numpy as np

N_CORES = 8
B_TOTAL = 16
B_PER_CORE = B_TOTAL // N_CORES
C = 256
H = 32
S = H * H          # 1024 spatial positions
NG = 32            # groupnorm groups -> 8 channels/group
NH = 4             # heads
CH = C // NH       # 64 channels/head
EPS = 1e-6
SCALE = CH ** -0.5  # 0.125

_CACHE: dict = {}


def _build_nc():
    from contextlib import ExitStack

    import concourse.bacc as bacc
    import concourse.bass as bass
    import concourse.mybir as mybir
    import concourse.tile as tile

    fp32 = mybir.dt.float32
    bf16 = mybir.dt.bfloat16
    i32 = mybir.dt.int32
    AF = mybir.ActivationFunctionType
    OP = mybir.AluOpType
    ts = bass.ts

    nc = bacc.Bacc("TRN2")

    x_d = nc.dram_tensor("x", [B_PER_CORE, C, S], fp32, kind="ExternalInput")
    wq_d = nc.dram_tensor("wq", [128, 2, C], bf16, kind="ExternalInput")
    wk_d = nc.dram_tensor("wk", [128, 2, C], bf16, kind="ExternalInput")
    wv_d = nc.dram_tensor("wv", [128, 2, C + NH], bf16, kind="ExternalInput")
    w3_d = nc.dram_tensor("w3", [128, 2, C], bf16, kind="ExternalInput")
    b3_d = nc.dram_tensor("b3p", [C], fp32, kind="ExternalInput")
    gns_d = nc.dram_tensor("gn_scale", [C], fp32, kind="ExternalInput")
    gnb_d = nc.dram_tensor("gn_bias", [C], fp32, kind="ExternalInput")
    y_d = nc.dram_tensor("y", [B_PER_CORE, C, S], fp32, kind="ExternalOutput")

    with tile.TileContext(nc) as tc, ExitStack() as ctx:
        const = ctx.enter_context(tc.tile_pool(name="const", bufs=1))
        xpool = ctx.enter_context(tc.tile_pool(name="xpool", bufs=1))
        opool = ctx.enter_context(tc.tile_pool(name="opool", bufs=2))
        hpool = ctx.enter_context(tc.tile_pool(name="hpool", bufs=2))
        qkpool = ctx.enter_context(tc.tile_pool(name="qkpool", bufs=1))
        vpool = ctx.enter_context(tc.tile_pool(name="vpool", bufs=16))
        gpool = ctx.enter_context(tc.tile_pool(name="gpool", bufs=16))
        epool = ctx.enter_context(tc.tile_pool(name="epool", bufs=6))
        upool = ctx.enter_context(tc.tile_pool(name="upool", bufs=2))
        npool = ctx.enter_context(tc.tile_pool(name="npool", bufs=4))
        spool = ctx.enter_context(tc.tile_pool(name="spool", bufs=2))
        ps = ctx.enter_context(tc.tile_pool(name="ps", bufs=1, space="PSUM"))

        # ---- phase 0: loads + constants ----
        x_sb = [[None, None], [None, None]]
        for ct in range(2):
            t = xpool.tile([128, S], fp32, tag=f"x0{ct}")
            nc.sync.dma_start(out=t, in_=x_d[0, ts(ct, 128), :])
            x_sb[0][ct] = t

        wq = const.tile([128, 2, C], bf16, tag="wq")
        nc.sync.dma_start(out=wq, in_=wq_d[:, :, :])
        wk = const.tile([128, 2, C], bf16, tag="wk")
        nc.sync.dma_start(out=wk, in_=wk_d[:, :, :])
        wv = const.tile([128, 2, C + NH], bf16, tag="wv")
        nc.sync.dma_start(out=wv, in_=wv_d[:, :, :])

        for ct in range(2):
            t = xpool.tile([128, S], fp32, tag=f"x1{ct}")
            nc.sync.dma_start(out=t, in_=x_d[1, ts(ct, 128), :])
            x_sb[1][ct] = t

        w3 = const.tile([128, 2, C], bf16, tag="w3")
        nc.sync.dma_start(out=w3, in_=w3_d[:, :, :])

        def col_tiles(dram, name):
            out = []
            for ct in range(2):
                t = const.tile([128, 1], fp32, tag=f"{name}{ct}")
                nc.sync.dma_start(out=t, in_=dram[ts(ct, 128)][:, None])
                out.append(t)
            return out

        gns_sb = col_tiles(gns_d, "gns")
        gnb_sb = col_tiles(gnb_d, "gnb")
        b3_sb = col_tiles(b3_d, "b3")

        # dummy exp: forces the single ACT table load before the stream
        dmy = spool.tile([1, 8], fp32, tag="dmy")
        nc.vector.memset(dmy, 0.0)
        nc.scalar.activation(out=dmy, in_=dmy, func=AF.Exp, scale=1.0)

        # ones row for the K=1 denominator-broadcast matmul
        ones1 = const.tile([1, 64], bf16, tag="ones1")
        nc.gpsimd.memset(ones1, 1.0)

        # q1[ct] [128, 32]: q1[p, g] = 1 iff group(ct*128+p) == g  (p//8 == g-16ct)
        q1 = []
        for ct in range(2):
            t = const.tile([128, NG], fp32, tag=f"q1{ct}")
            nc.gpsimd.memset(t, 1.0)
            nc.gpsimd.affine_select(out=t, in_=t, compare_op=OP.is_ge, fill=0.0,
                                    pattern=[[-8, NG]], base=128 * ct,
                                    channel_multiplier=1)
            nc.gpsimd.affine_select(out=t, in_=t, compare_op=OP.is_ge, fill=0.0,
                                    pattern=[[8, NG]], base=7 - 128 * ct,
                                    channel_multiplier=-1)
            q1.append(t)

        # q2[ct] [32, 128]: q2[g, c] = 1 iff group(ct*128+c) == g
        q2 = []
        for ct in range(2):
            t = const.tile([NG, 128], fp32, tag=f"q2{ct}")
            nc.gpsimd.memset(t, 1.0)
            base = ct * 128
            nc.gpsimd.affine_select(out=t, in_=t, compare_op=OP.is_ge, fill=0.0,
                                    pattern=[[1, 128]], base=base,
                                    channel_multiplier=-8)
            nc.gpsimd.affine_select(out=t, in_=t, compare_op=OP.is_ge, fill=0.0,
                                    pattern=[[-1, 128]], base=7 - base,
                                    channel_multiplier=8)
            q2.append(t)

        # ---- phase 1 per batch: groupnorm + q/k/vT projections ----
        # E [65, 64]: row 64 = 1 else 0 (denominator-broadcast stationary)
        E = const.tile([65, 64], bf16, tag="E")
        nc.gpsimd.memset(E, 0.0)
        nc.gpsimd.memset(E[64:65, :], 1.0)

        qk_sb_all, vt_all, gsb_all = [], [], []

        def phase1(b):
            # stats per ct-tile (groups don't cross the ct boundary)
            gs_ps = ps.tile([NG, 2], fp32, tag="s0", bufs=1, name="gs_ps")
            for ct in range(2):
                st6 = spool.tile([128, 2, 6], fp32, tag="st6")
                for k in range(2):
                    nc.vector.bn_stats(out=st6[:, k, :],
                                       in_=x_sb[b][ct][:, ts(k, 512)])
                mv = spool.tile([128, 2], fp32, tag="mv")
                nc.vector.bn_aggr(out=mv, in_=st6)
                rhs2 = spool.tile([128, 2], fp32, tag="rhs2")
                nc.vector.tensor_copy(out=rhs2[:, 0:1], in_=mv[:, 0:1])
                nc.vector.tensor_mul(out=rhs2[:, 1:2], in0=mv[:, 0:1],
                                     in1=mv[:, 0:1])
                nc.vector.tensor_add(out=rhs2[:, 1:2], in0=rhs2[:, 1:2],
                                     in1=mv[:, 1:2])
                nc.tensor.matmul(gs_ps, lhsT=q1[ct], rhs=rhs2,
                                 start=(ct == 0), stop=(ct == 1))
            gmv = spool.tile([NG, 2], fp32, tag="gmv")
            nc.vector.tensor_scalar_mul(out=gmv, in0=gs_ps, scalar1=1.0 / 8.0)
            # v = var + eps
            v = spool.tile([NG, 1], fp32, tag="qv")
            nc.vector.tensor_mul(out=v, in0=gmv[:, 0:1], in1=gmv[:, 0:1])
            nc.vector.tensor_tensor(out=v, in0=gmv[:, 1:2], in1=v,
                                    op=OP.subtract)
            nc.vector.tensor_scalar_add(out=v, in0=v, scalar1=EPS)
            # quake rsqrt + 2 Newton steps
            t1 = spool.tile([NG, 1], i32, tag="qt1")
            nc.vector.tensor_scalar(out=t1, in0=v.bitcast(i32), scalar1=1,
                                    scalar2=0xFFFFFFFF,
                                    op0=OP.logical_shift_right,
                                    op1=OP.bitwise_xor)
            y0i = spool.tile([NG, 1], i32, tag="qy0")
            nc.vector.tensor_scalar(out=y0i, in0=t1, scalar1=0x5f3759e0,
                                    scalar2=None, op0=OP.add)
            ab_g = spool.tile([NG, 2], fp32, tag="abg")
            y2 = spool.tile([NG, 1], fp32, tag="qy2")
            t3 = spool.tile([NG, 1], fp32, tag="qt3")
            y1 = spool.tile([NG, 1], fp32, tag="qy1")
            cur = y0i.bitcast(fp32)
            for it in range(1, 2):
                nc.vector.tensor_mul(out=y2, in0=cur, in1=cur)
                nc.vector.tensor_mul(out=t3, in0=y2, in1=v)
                nc.vector.tensor_scalar(out=t3, in0=t3, scalar1=-0.5,
                                        scalar2=1.5, op0=OP.mult, op1=OP.add)
                dst = ab_g[:, 0:1] if it == 1 else y1
                nc.vector.tensor_mul(out=dst, in0=cur, in1=t3)
                cur = dst
            nc.vector.tensor_mul(out=ab_g[:, 1:2], in0=gmv[:, 0:1],
                                 in1=ab_g[:, 0:1])
            nc.vector.tensor_scalar_mul(out=ab_g[:, 1:2], in0=ab_g[:, 1:2],
                                        scalar1=-1.0)

            h_bf = []
            for ct in range(2):
                ab_ps = ps.tile([128, 2], fp32, tag="s1", bufs=1, name="ab_ps")
                nc.tensor.matmul(ab_ps, lhsT=q2[ct], rhs=ab_g, start=True,
                                 stop=True)
                AB = spool.tile([128, 2], fp32, tag=f"AB{ct}")
                nc.vector.tensor_mul(out=AB[:, 0:1], in0=ab_ps[:, 0:1],
                                     in1=gns_sb[ct])
                nc.vector.tensor_mul(out=AB[:, 1:2], in0=ab_ps[:, 1:2],
                                     in1=gns_sb[ct])
                nc.vector.tensor_add(out=AB[:, 1:2], in0=AB[:, 1:2],
                                     in1=gnb_sb[ct])
                ht = hpool.tile([128, S], bf16, tag=f"h{ct}")
                nc.vector.tensor_scalar(out=ht, in0=x_sb[b][ct],
                                        scalar1=AB[:, 0:1], scalar2=AB[:, 1:2],
                                        op0=OP.mult, op1=OP.add)
                h_bf.append(ht)
            # q/k projections -> bf16 [d_tile 128, s 1024]
            qk_sb = [[None, None], [None, None]]
            for p, wt in ((0, wq), (1, wk)):
                for dt in range(2):
                    qk_ps = ps.tile([128, S], fp32, tag=f"s{(2 * p + dt) % 2}",
                                    bufs=1, name="qk_ps")
                    for sc in range(2):
                        for ct in range(2):
                            nc.tensor.matmul(
                                qk_ps[:, ts(sc, 512)],
                                lhsT=wt[:, ct, ts(dt, 128)],
                                rhs=h_bf[ct][:, ts(sc, 512)],
                                start=(ct == 0), stop=(ct == 1))
                    t = qkpool.tile([128, S], bf16, tag=f"qk{b}{p}{dt}")
                    nc.scalar.copy(out=t, in_=qk_ps)
                    qk_sb[p][dt] = t

            # vT tiles [128 kp, 4 heads, 65] + per-head g columns
            vt_tiles, gsb_tiles = [], []
            for j in range(8):
                vt_ps = ps.tile([128, C + NH], fp32, tag=f"s{j % 2}",
                                bufs=1, name="vt_ps")
                for ct in range(2):
                    nc.tensor.matmul(vt_ps, lhsT=h_bf[ct][:, ts(j, 128)],
                                     rhs=wv[:, ct, :], start=(ct == 0),
                                     stop=(ct == 1))
                vt = vpool.tile([128, NH, CH + 1], bf16, tag="vt")
                nc.gpsimd.memset(vt[:, :, CH:CH + 1], 1.0)
                nc.vector.tensor_copy(
                    out=vt[:, :, 0:CH],
                    in_=vt_ps[:, 0:C].rearrange("p (h c) -> p h c", h=NH))
                gsb = gpool.tile([128, NH], fp32, tag="gsb")
                nc.vector.tensor_scalar_mul(out=gsb, in0=vt_ps[:, C:C + NH],
                                            scalar1=SCALE)
                vt_tiles.append(vt)
                gsb_tiles.append(gsb)
            qk_sb_all.append(qk_sb)
            vt_all.append(vt_tiles)
            gsb_all.append(gsb_tiles)

        phase1(0)
        phase1(1)

        # ---- phase 2: attention pr-blocks, software-pipelined ----
        blocks = [(0, 0), (0, 1), (1, 0), (1, 1)]
        hh_n_all = {}

        def scores_exp(b, pr, j, hp, s_tiles):
            qk_sb = qk_sb_all[b]
            s_ps = ps.tile([128, S], fp32, tag=f"s{hp}", bufs=1, name="s_ps")
            for sc in range(2):
                nc.tensor.matmul(
                    s_ps[:, ts(sc, 512)],
                    lhsT=qk_sb[1][pr][ts(hp, CH), ts(j, 128)],
                    rhs=qk_sb[0][pr][ts(hp, CH), ts(sc, 512)],
                    start=True, stop=True)
            et = epool.tile([128, S], bf16, tag="e")
            nc.scalar.activation(out=et, in_=s_ps, func=AF.Exp,
                                 bias=gsb_all[b][j][:, 2 * pr + hp:2 * pr + hp + 1],
                                 scale=SCALE)
            s_tiles[(j, hp)] = et

        def av(b, pr, j, hp, s_tiles, hh_ps):
            vt = vt_all[b][j]
            et = s_tiles[(j, hp)]
            for sc in range(2):
                nc.tensor.matmul(hh_ps[hp][sc],
                                 lhsT=vt[:, 2 * pr + hp, :],
                                 rhs=et[:, ts(sc, 512)],
                                 start=(j == 0), stop=(j == 7))

        def normalize(b, pr, hp, hh_ps):
            # bf16 eviction feeds the E-matmul denom broadcast; numerator
            # stays fp32 in PSUM for the final multiply.
            hh_b = upool.tile([CH + 1, S], bf16, tag="hhb")
            rd = upool.tile([CH, S], fp32, tag="rd")
            hh_n = hh_n_all[(b, pr)]
            for sc in range(2):
                nc.vector.tensor_copy(out=hh_b[:, ts(sc, 512)],
                                      in_=hh_ps[hp][sc])
            db_ps = [ps.tile([CH, 512], fp32, tag=f"s{sc}", bufs=1,
                             name=f"db{sc}") for sc in range(2)]
            for sc in range(2):
                nc.tensor.matmul(db_ps[sc], lhsT=E,
                                 rhs=hh_b[:, ts(sc, 512)], start=True,
                                 stop=True)
            for sc in range(2):
                nc.vector.reciprocal_approx_fast(out=rd[:, ts(sc, 512)],
                                                 in_=db_ps[sc])
            for sc in range(2):
                nc.vector.tensor_mul(out=hh_n[ts(hp, CH), ts(sc, 512)],
                                     in0=hh_ps[hp][sc][0:CH, :],
                                     in1=rd[:, ts(sc, 512)])

        def fin(b, dt):
            # final nin + residual + output DMA for one d-tile
            fin_ps = ps.tile([128, S], fp32, tag=f"s{dt}", bufs=1,
                             name="fin_ps")
            for sc in range(2):
                for ct in range(2):
                    nc.tensor.matmul(
                        fin_ps[:, ts(sc, 512)],
                        lhsT=w3[:, ct, ts(dt, 128)],
                        rhs=hh_n_all[(b, ct)][:, ts(sc, 512)],
                        start=(ct == 0), stop=(ct == 1))
            out_t = opool.tile([128, S], fp32, tag=f"out{dt}")
            nc.vector.scalar_tensor_tensor(out=out_t, in0=fin_ps,
                                           scalar=b3_sb[dt], in1=x_sb[b][dt],
                                           op0=OP.add, op1=OP.add)
            nc.sync.dma_start(out=y_d[b, ts(dt, 128), :], in_=out_t)

        prev = None
        prev_state = None
        for b, pr in blocks:
            hh_n_all[(b, pr)] = npool.tile([128, S], bf16, tag="hhn", name=f"hhn{b}{pr}")
            s_tiles = {}
            hh_ps = None
            for j in range(8):
                # lagged work from the previous block first (keeps the PSUM
                # WAR order correct: prev readers emitted before cur writers)
                if prev is not None:
                    pb, ppr = prev
                    if j == 0:
                        for hp in range(2):
                            av(pb, ppr, 7, hp, prev_state[0], prev_state[1])
                    elif j == 1:
                        for hp in range(2):
                            normalize(pb, ppr, hp, prev_state[1])
                    elif j == 2 and ppr == 1:
                        fin(pb, 0)
                    elif j == 3 and ppr == 1:
                        fin(pb, 1)
                for hp in range(2):
                    scores_exp(b, pr, j, hp, s_tiles)
                if j >= 1:
                    for hp in range(2):
                        f_ps = ps.tile([128, 512], fp32, tag=f"s{hp}",
                                       bufs=1, name="f_ps")
                        nc.tensor.matmul(
                            f_ps, lhsT=qk_sb_all[b][0][pr][:, 0:128],
                            rhs=qk_sb_all[b][0][pr][:, 0:512],
                            start=True, stop=True)
                if j == 1:
                    hh_ps = [[ps.tile([CH + 1, 512], fp32, tag="hh", bufs=4,
                                      name=f"hh{hp}{sc}") for sc in range(2)]
                             for hp in range(2)]
                if j >= 1:
                    for hp in range(2):
                        av(b, pr, j - 1, hp, s_tiles, hh_ps)
            prev = (b, pr)
            prev_state = (s_tiles, hh_ps)

        # epilogue: last block's tail
        pb, ppr = prev
        for hp in range(2):
            av(pb, ppr, 7, hp, prev_state[0], prev_state[1])
        for hp in range(2):
            normalize(pb, ppr, hp, prev_state[1])
        fin(pb, 0)
        fin(pb, 1)

    nc.finalize()
    return nc


def _pack_weights(inputs):
    """Host-side algebra + bf16 packing. Returns dict of shared arrays."""
    import ml_dtypes

    W0 = np.asarray(inputs["W0"], np.float32)
    b0 = np.asarray(inputs["b0"], np.float32)
    W1 = np.asarray(inputs["W1"], np.float32)
    W2 = np.asarray(inputs["W2"], np.float32)
    b2 = np.asarray(inputs["b2"], np.float32)
    W3 = np.asarray(inputs["W3"], np.float32)
    b3 = np.asarray(inputs["b3"], np.float32)

    def pack(w):
        # [C, D] -> [128, 2, D] with c = ct*128 + p
        return np.ascontiguousarray(
            w.reshape(2, 128, -1).transpose(1, 0, 2)).astype(ml_dtypes.bfloat16)

    G = np.zeros((C, NH), np.float32)
    for h in range(NH):
        G[:, h] = W1[:, h * CH:(h + 1) * CH] @ b0[h * CH:(h + 1) * CH]
    wv_ext = np.concatenate([W2, G], axis=1)          # [C, 260]
    b3p = b3 + W3.T @ b2

    return {
        "wq": pack(W0),
        "wk": pack(W1),
        "wv": pack(wv_ext),
        "w3": pack(W3),
        "b3p": np.ascontiguousarray(b3p, np.float32),
        "gn_scale": np.ascontiguousarray(np.asarray(inputs["gn_scale"], np.float32)),
        "gn_bias": np.ascontiguousarray(np.asarray(inputs["gn_bias"], np.float32)),
    }


def _in_maps(inputs):
    x = np.ascontiguousarray(np.asarray(inputs["x"], dtype=np.float32))
    B = x.shape[0]
    xr = x.reshape(B, C, S)
    shared = _pack_weights(inputs)
    maps = []
    for core in range(N_CORES):
        m = dict(shared)
        m["x"] = np.ascontiguousarray(xr[core * B_PER_CORE:(core + 1) * B_PER_CORE])
        maps.append(m)
    return maps


def kernel(**inputs: np.ndarray) -> np.ndarray:
    from concourse.bass_utils import run_bass_kernel_spmd

    if "nc" not in _CACHE:
        _CACHE["nc"] = _build_nc()
    res = run_bass_kernel_spmd(_CACHE["nc"], _in_maps(inputs),
                               core_ids=list(range(N_CORES)))
    out = np.concatenate([res.results[c]["y"] for c in range(N_CORES)], axis=0)
    B = np.asarray(inputs["x"]).shape[0]
    return out.reshape(B, C, H, H).astype(np.float32)


def run_profiled(inputs):
    """Like kernel() but with trace=True; returns (out, exec_time_ns)."""
    from concourse.bass_utils import run_bass_kernel_spmd

    if "nc" not in _CACHE:
        _CACHE["nc"] = _build_nc()
    res = run_bass_kernel_spmd(_CACHE["nc"], _in_maps(inputs),
                               core_ids=list(range(N_CORES)), trace=True)
    out = np.concatenate([res.results[c]["y"] for c in range(N_CORES)], axis=0)
    B = np.asarray(inputs["x"]).shape[0]
    return out.reshape(B, C, H, H).astype(np.float32), res.exec_time_ns
